# revision 10
# baseline (speedup 1.0000x reference)
"""Log-sparse attention kernel for 8 TRN2 NeuronCores.

Sharding: batch (4) x head-group (2 groups of 4 heads) = 8 cores, no
collectives.  Each core computes its batch's QK causal-conv projection for
its 4 heads, transposed scores ST[kj,qi] (float32r matmuls), exp on ScalarE
straight out of PSUM (1/8 scale folded in), multiplicative log-sparse mask in
bf16 on VectorE, attention*V with a ones-column in V for free row-sums,
normalized attention weights emitted via PE transpose + fused
scale-on-copyback, and its partial output projection.  The host sums the two
per-batch projection partials and adds bp.

Only the causal lower triangle is computed; the PJRT path zero-initializes
output buffers so the upper triangle is never written.
"""

from contextlib import ExitStack

import numpy as np
import ml_dtypes

import concourse.bacc as bacc
import concourse.tile as tile
import concourse.mybir as mybir

B, T, E, H, QL, SUB = 4, 2048, 64, 8, 6, 64
HG = 4          # heads per core
NCORES = 8
NT = T // 128   # 16 blocks of 128 along t/qi/kj

F32 = mybir.dt.float32
F32R = mybir.dt.float32r
BF16 = mybir.dt.bfloat16
BNP = ml_dtypes.bfloat16

def _qi0(kb):
    """First qi column computed for kj-block kb (causal block sparsity,
    rounded down to the 512-wide matmul tile)."""
    return (kb // 4) * 512


_MT_OFF = []
_o = 0
for _kb in range(NT):
    _MT_OFF.append(_o)
    _o += T - _qi0(_kb)
MT_W = _o  # 20480


def _build_nc():
    nc = bacc.Bacc("TRN2", target_bir_lowering=False, debug=False,
                   num_devices=NCORES)
    xT = nc.dram_tensor("xT", [E + 1, T + QL - 1], F32R, kind="ExternalInput").ap()
    wqk = nc.dram_tensor("wqk", [E + 1, QL, 2 * HG * E], F32R, kind="ExternalInput").ap()
    wv = nc.dram_tensor("wv", [E + 1, HG * (E + 1)], F32R, kind="ExternalInput").ap()
    wp = nc.dram_tensor("wp", [E, HG * E], BF16, kind="ExternalInput").ap()
    mt = nc.dram_tensor("mt", [128, MT_W], BF16, kind="ExternalInput").ap()
    ident = nc.dram_tensor("ident", [128, 128], BF16, kind="ExternalInput").ap()
    w_out = nc.dram_tensor("w", [HG, T, T], F32, kind="ExternalOutput").ap()
    po_out = nc.dram_tensor("po", [T, E], F32, kind="ExternalOutput").ap()

    Exp = mybir.ActivationFunctionType.Exp

    with tile.TileContext(nc) as tc, ExitStack() as ctx:
        const = ctx.enter_context(tc.tile_pool(name="const", bufs=1))
        qk_pool = ctx.enter_context(tc.tile_pool(name="qkT", bufs=1))
        v_pool = ctx.enter_context(tc.tile_pool(name="vall", bufs=1))
        pmt_pools = {
            w: ctx.enter_context(tc.tile_pool(name=f"pmt{w}", bufs=5))
            for w in (2048, 1536, 1024, 512)
        }
        w_pool = ctx.enter_context(tc.tile_pool(name="wrow", bufs=2))
        o_pool = ctx.enter_context(tc.tile_pool(name="osb", bufs=4))
        ot_pool = ctx.enter_context(tc.tile_pool(name="otsb", bufs=1))
        r_pool = ctx.enter_context(tc.tile_pool(name="rinv", bufs=2))
        po_pool = ctx.enter_context(tc.tile_pool(name="posb", bufs=1))

        # ---- load inputs -------------------------------------------------
        xT_sb = const.tile([E + 1, T + QL - 1], F32R)
        nc.sync.dma_start(xT_sb[:], xT[:])
        wqk_sb = const.tile([E + 1, QL, 2 * HG * E], F32R)
        nc.sync.dma_start(wqk_sb[:], wqk[:])
        wv_sb = const.tile([E + 1, HG * (E + 1)], F32R)
        nc.sync.dma_start(wv_sb[:], wv[:])
        wp_sb = const.tile([E, HG * E], BF16)
        nc.sync.dma_start(wp_sb[:], wp[:])
        ident_sb = const.tile([128, 128], BF16)
        nc.sync.dma_start(ident_sb[:], ident[:])
        mt_sb = const.tile([128, MT_W], BF16)
        nc.sync.dma_start(mt_sb[:], mt[:])

        # ---- causal-conv QK projection: qkT[c, t], c in [0,512) ----------
        # channels: 0..255 = q of heads 0..3 (64 each); 256..511 = k.
        conv_ctx = ExitStack()
        ps_conv = conv_ctx.enter_context(
            tc.tile_pool(name="psc", bufs=2, space="PSUM"))
        qkT_sb = qk_pool.tile([128, 4, T], BF16)
        for ct in range(4):
            for tt in range(4):
                cps = ps_conv.tile([128, 512], F32)
                for dt in range(QL):
                    nc.tensor.matmul(
                        cps[:, :],
                        lhsT=wqk_sb[:, dt, ct * 128:(ct + 1) * 128],
                        rhs=xT_sb[:, tt * 512 + dt: tt * 512 + dt + 512],
                        start=(dt == 0), stop=(dt == QL - 1),
                    )
                nc.scalar.copy(qkT_sb[:, ct, tt * 512:(tt + 1) * 512], cps[:, :])

        # ---- value projection with ones-column: v_all[p, tb, 260] bf16 ---
        v_sb = v_pool.tile([128, NT, HG * (E + 1)], BF16)
        for tb in range(NT):
            vps = ps_conv.tile([128, HG * (E + 1)], F32, tag="vps")
            nc.tensor.matmul(
                vps[:, :],
                lhsT=xT_sb[:, QL - 1 + tb * 128: QL - 1 + (tb + 1) * 128],
                rhs=wv_sb[:],
                start=True, stop=True,
            )
            nc.vector.tensor_copy(v_sb[:, tb, :], vps[:, :])
        conv_ctx.close()

        main_ctx = ExitStack()
        ps_st = main_ctx.enter_context(
            tc.tile_pool(name="psst", bufs=2, space="PSUM"))
        ps_av = main_ctx.enter_context(
            tc.tile_pool(name="psav", bufs=2, space="PSUM"))
        ps_tr = main_ctx.enter_context(
            tc.tile_pool(name="pstr", bufs=2, space="PSUM"))
        o_tiles = []
        for hh in range(HG):
            # q rows of head hh live at qkT_sb[64*(hh%2):+64, hh//2, :]
            # k rows at qkT_sb[64*(hh%2):+64, 2 + hh//2, :]
            p0 = 64 * (hh % 2)
            qct = hh // 2
            kct = 2 + hh // 2

            # ---- phase A: ST = K^T Q (transposed scores), exp, mask ------
            pmt_tiles = []
            for kb in range(NT):
                q0 = _qi0(kb)
                wdt = T - q0
                pmt_t = pmt_pools[wdt].tile([128, wdt], BF16)
                for seg in range(q0, T, 1024):
                    segw = min(1024, T - seg)
                    st_t = ps_st.tile([128, 1024], F32)
                    for off in range(0, segw, 512):
                        nc.tensor.matmul(
                            st_t[:, off:off + 512],
                            lhsT=qkT_sb[p0:p0 + 64, kct,
                                        kb * 128:(kb + 1) * 128],
                            rhs=qkT_sb[p0:p0 + 64, qct,
                                       seg + off: seg + off + 512],
                            start=True, stop=True,
                        )
                    nc.scalar.activation(
                        pmt_t[:, seg - q0: seg - q0 + segw],
                        st_t[:, 0:segw], Exp, scale=0.125)
                nc.vector.tensor_mul(
                    pmt_t[:, :], pmt_t[:, :],
                    mt_sb[:, _MT_OFF[kb]: _MT_OFF[kb] + wdt])
                pmt_tiles.append(pmt_t)

            # ---- phase B/C per qi-block: AV, rowsums, W output -----------
            o_t = o_pool.tile([128, NT, E], BF16)
            rinv_t = r_pool.tile([128, NT], F32)
            o_tiles.append(o_t)
            for qb in range(NT):
                av = ps_av.tile([128, E + 1], F32)
                for kb in range(qb + 1):
                    nc.tensor.matmul(
                        av[:, :],
                        lhsT=pmt_tiles[kb][:, qb * 128 - _qi0(kb):
                                           qb * 128 - _qi0(kb) + 128],
                        rhs=v_sb[:, kb, hh * (E + 1): (hh + 1) * (E + 1)],
                        start=(kb == 0), stop=(kb == qb),
                    )
                nc.vector.reciprocal(rinv_t[:, qb:qb + 1], av[:, E:E + 1])
                nc.vector.tensor_scalar_mul(
                    o_t[:, qb, :], av[:, 0:E], rinv_t[:, qb:qb + 1])

                wrow = w_pool.tile([128, T], F32)
                nchunk = 0
                for c0 in range(0, (qb + 1) * 128, 1024):
                    cw = min(1024, (qb + 1) * 128 - c0)
                    tr = ps_tr.tile([128, 1024], BF16)
                    for j in range(0, cw, 128):
                        kbj = (c0 + j) // 128
                        nc.tensor.transpose(
                            tr[:, j:j + 128],
                            pmt_tiles[kbj][:, qb * 128 - _qi0(kbj):
                                           qb * 128 - _qi0(kbj) + 128],
                            ident_sb[:])
                    if nchunk % 2 == 0:
                        nc.vector.tensor_scalar_mul(
                            wrow[:, c0:c0 + cw], tr[:, 0:cw],
                            rinv_t[:, qb:qb + 1])
                    else:
                        nc.scalar.mul(
                            wrow[:, c0:c0 + cw], tr[:, 0:cw],
                            rinv_t[:, qb:qb + 1])
                    nchunk += 1
                nc.sync.dma_start(
                    w_out[hh, qb * 128:(qb + 1) * 128, 0:(qb + 1) * 128],
                    wrow[:, 0:(qb + 1) * 128])

        main_ctx.close()

        # ---- transpose O to [e_head, t] then project ---------------------
        end_ctx = ExitStack()
        ps_end = end_ctx.enter_context(
            tc.tile_pool(name="psend", bufs=2, space="PSUM"))
        ot_sb = ot_pool.tile([E, HG, T], BF16)
        for hh in range(HG):
            for tc4 in range(4):
                otr = ps_end.tile([E, 512], BF16)
                for j in range(4):
                    tb = tc4 * 4 + j
                    nc.tensor.transpose(
                        otr[:, j * 128:(j + 1) * 128],
                        o_tiles[hh][:, tb, :], ident_sb[:])
                nc.vector.tensor_copy(ot_sb[:, hh, tc4 * 512:(tc4 + 1) * 512],
                                      otr[:, :])

        po_sb = po_pool.tile([128, NT, E], F32)
        for tb in range(NT):
            pp = ps_end.tile([128, E], F32, tag="pp")
            for hh in range(HG):
                nc.tensor.matmul(
                    pp[:, :],
                    lhsT=ot_sb[:, hh, tb * 128:(tb + 1) * 128],
                    rhs=wp_sb[:, hh * E:(hh + 1) * E],
                    start=(hh == 0), stop=(hh == HG - 1),
                )
            nc.vector.tensor_copy(po_sb[:, tb, :], pp[:, :])
        end_ctx.close()
        nc.sync.dma_start(
            po_out.rearrange("(n p) e -> p n e", p=128), po_sb[:])

    nc.compile()
    return nc


# --------------------------------------------------------------------------
# Host-side sharding / unsharding and the cached PJRT runner.
# --------------------------------------------------------------------------

_RUNNER = None


class _Runner:
    def __init__(self):
        import jax
        from jax.experimental.shard_map import shard_map
        from jax.sharding import Mesh, PartitionSpec
        from concourse.bass2jax import (
            _bass_exec_p, install_neuronx_cc_hook, partition_id_tensor)

        install_neuronx_cc_hook()
        nc = self.nc = _build_nc()
        partition_name = (
            nc.partition_id_tensor.name if nc.partition_id_tensor else None)

        in_names, out_names, out_avals, zero_shapes = [], [], [], []
        for alloc in nc.m.functions[0].allocations:
            if not isinstance(alloc, mybir.MemoryLocationSet):
                continue
            name = alloc.memorylocations[0].name
            if alloc.kind == "ExternalInput":
                if name != partition_name:
                    in_names.append(name)
            elif alloc.kind == "ExternalOutput":
                out_names.append(name)
                shape = tuple(alloc.tensor_shape)
                dtype = mybir.dt.np(alloc.dtype)
                out_avals.append(jax.core.ShapedArray(shape, dtype))
                zero_shapes.append((shape, dtype))
        self.in_names = in_names
        self.out_names = out_names
        self.out_avals = out_avals
        self.zero_shapes = zero_shapes
        n_params = len(in_names)
        n_outs = len(out_names)
        all_names = in_names + out_names
        if partition_name is not None:
            all_names = all_names + [partition_name]

        def _body(*args):
            operands = list(args)
            if partition_name is not None:
                operands.append(partition_id_tensor())
            outs = _bass_exec_p.bind(
                *operands,
                out_avals=tuple(out_avals),
                in_names=tuple(all_names),
                out_names=tuple(out_names),
                lowering_input_output_aliases=(),
                sim_require_finite=True,
                sim_require_nnan=True,
                nc=nc,
            )
            return tuple(outs)

        devices = jax.devices()[:NCORES]
        mesh = Mesh(np.asarray(devices), ("core",))
        in_specs = (PartitionSpec("core"),) * (n_params + n_outs)
        out_specs = (PartitionSpec("core"),) * n_outs
        donate = tuple(range(n_params, n_params + n_outs))
        self.sharded = jax.jit(
            shard_map(_body, mesh=mesh, in_specs=in_specs,
                      out_specs=out_specs, check_rep=False),
            donate_argnums=donate, keep_unused=True,
        )

    def execute(self, in_maps):
        concat_in = [
            np.concatenate([np.asarray(m[name]) for m in in_maps], axis=0)
            for name in self.in_names
        ]
        concat_zeros = [
            np.zeros((NCORES * s[0], *s[1:]), d) for (s, d) in self.zero_shapes
        ]
        out_arrs = self.sharded(*concat_in, *concat_zeros)
        return [
            {
                name: np.asarray(out_arrs[i]).reshape(
                    NCORES, *self.out_avals[i].shape)[c]
                for i, name in enumerate(self.out_names)
            }
            for c in range(NCORES)
        ]


def _get_runner():
    global _RUNNER
    if _RUNNER is None:
        _RUNNER = _Runner()
    return _RUNNER


def _prep_in_maps(x, Wqk, bqk, Wv, bv, Wp, bp, mask):
    x = np.asarray(x, np.float32)
    Wqk = np.asarray(Wqk, np.float32)
    bqk = np.asarray(bqk, np.float32)
    Wv = np.asarray(Wv, np.float32)
    bv = np.asarray(bv, np.float32)
    Wp = np.asarray(Wp, np.float32)
    m = np.asarray(mask).reshape(T, T).astype(np.float32)

    # mask, transposed + causally packed (shared by all cores)
    mt_np = np.zeros((128, MT_W), dtype=BNP)
    for kb in range(NT):
        q0 = _qi0(kb)
        blk = m[q0:, kb * 128:(kb + 1) * 128].T  # [128, T-q0]
        mt_np[:, _MT_OFF[kb]: _MT_OFF[kb] + T - q0] = blk.astype(BNP)
    ident_np = np.eye(128, dtype=BNP)

    in_maps = []
    for core in range(NCORES):
        b, g = divmod(core, 2)
        heads = [4 * g + i for i in range(HG)]

        xT_np = np.zeros((E + 1, T + QL - 1), np.float32)
        xT_np[:E, QL - 1:] = x[b].T
        xT_np[E, :] = 1.0

        # channel order: q of the 4 heads (64 each), then k of the 4 heads
        chan = np.concatenate(
            [np.arange(64 * h, 64 * h + 64) for h in heads]
            + [np.arange(512 + 64 * h, 512 + 64 * h + 64) for h in heads])
        wqk_np = np.zeros((E + 1, QL, 2 * HG * E), np.float32)
        # Wqk: [1024, 64, 6] -> [e, dt, ci]
        wqk_np[:E] = Wqk[chan].transpose(1, 2, 0)
        wqk_np[E, 0, :] = bqk[chan]

        wv_np = np.zeros((E + 1, HG * (E + 1)), np.float32)
        for i, h in enumerate(heads):
            wv_np[:E, i * (E + 1): i * (E + 1) + E] = Wv[:, 64 * h: 64 * h + 64]
            wv_np[E, i * (E + 1): i * (E + 1) + E] = bv[64 * h: 64 * h + 64]
            wv_np[E, i * (E + 1) + E] = 1.0

        wp_np = np.zeros((E, HG * E), np.float32)
        for i, h in enumerate(heads):
            wp_np[:, i * E:(i + 1) * E] = Wp[64 * h: 64 * h + 64, :]

        in_maps.append({
            "xT": xT_np,
            "wqk": wqk_np,
            "wv": wv_np,
            "wp": wp_np.astype(BNP),
            "mt": mt_np,
            "ident": ident_np,
        })
    return in_maps


def _assemble(results, bp):
    bp = np.asarray(bp, np.float32)
    attn = np.empty((B, H, T, T), np.float32)
    out = np.empty((B, T, E), np.float32)
    for core in range(NCORES):
        b, g = divmod(core, 2)
        attn[b, 4 * g: 4 * g + HG] = results[core]["w"]
    for b in range(B):
        out[b] = results[2 * b]["po"] + results[2 * b + 1]["po"] + bp
    return out, attn


def kernel(x, Wqk, bqk, Wv, bv, Wp, bp, mask):
    runner = _get_runner()
    in_maps = _prep_in_maps(x, Wqk, bqk, Wv, bv, Wp, bp, mask)
    results = runner.execute(in_maps)
    return _assemble(results, bp)
